# revision 5
# baseline (speedup 1.0000x reference)
"""Trainium2 Bass kernel for nn_Attention_53188874993896 (sparse_attention).

Math notes (derived from the reference):
  - pos_scores[b,h,s,t] = r[b,h,s] - r[b,h,t] + head_b[h] with r = p @ head_w[h].
    The s-dependent part is constant along the softmax axis t, so
    pos_attn[b,h,s,t] = w[b,h,t] where w = softmax_t(-r): rank-1 collapse.
  - head_b and the pos_b2-induced shift are constant along t -> softmax-
    invariant -> dropped; p enters only via head_w @ p, so pos_w2/head_w fold
    into one host-side [PD, H] matrix (p is never materialized).
  - blend a = (1-g)*attn + g*pos_attn already has rows summing to 1, so the
    reference's renormalization is an identity up to float rounding.
  - softmax without max-subtraction is safe: |scores| <~ 4.

Implementation (cost model: matmul time ~ out-free-size * pe_cycle * cpr):
  - x / pos are uploaded pre-transposed bf16 (host-side layout staging),
    weights packed into 2 DMAs; all weights resident in SBUF.
  - scores: fp8e4m3 DoubleRow matmuls with a stride-0 k-tile (each product
    summed twice, the 2x folded into the exp scale): 0.5 cy/row.
  - ctx computed transposed (out[s_part, (h,j)], free=32/head) with free-1
    den column matmuls; normalize+transpose back via PE; vbar via 16 free-1
    matmuls using per-core-prenormalized w columns; pos row + bias enter the
    final matmul as rank-1 terms.
  - Software pipeline: per batch, A = qkv+scores+exp, B = ctx..final+store;
    B(b-2) is emitted piecewise inside A(b) (skew-2) so the ACT exp stream
    (the binding resource, ~4.16us/batch) runs nearly gapless. Scores PSUM
    is split into two 2-bank ping-pong slots to hide the exp round-trip.
  - PE pstate warmup dummies + act-table preload at t=0; out-stores on the
    Pool SWDGE queue (last two on HWDGE to shorten the drain).

Sharding: data-parallel over batch B=64 across 8 cores (8 batches/core).
"""

import sys

sys.path.insert(0, "/opt/trn_rl_repo")

import numpy as np
import ml_dtypes

B, S, D, H, PD = 64, 256, 256, 8, 8
HD = D // H  # 32
P8 = D // 8  # 32
NCORES = 8
NB = B // NCORES  # batches per core
SCALE = 1.0 / np.sqrt(np.float32(HD))

bf16 = ml_dtypes.bfloat16
f8e4 = ml_dtypes.float8_e4m3

FP8_SCORES = True

_CACHE = {}


def _build(nb, fp8=FP8_SCORES):
    import concourse.bass as bass
    import concourse.bacc as bacc
    import concourse.mybir as mybir
    from concourse.tile import TileContext

    fp32 = mybir.dt.float32
    bf = mybir.dt.bfloat16
    f8 = mybir.dt.float8e4
    sdt = f8 if fp8 else bf
    Exp = mybir.ActivationFunctionType.Exp
    Relu = mybir.ActivationFunctionType.Relu
    Copy = mybir.ActivationFunctionType.Copy
    DR = mybir.MatmulPerfMode.DoubleRow

    nc = bacc.Bacc("TRN2", target_bir_lowering=False, debug=False)

    # ---- DRAM I/O ----
    xt_d = nc.dram_tensor("xT", [nb, D, S], bf, kind="ExternalInput")
    wp5_d = nc.dram_tensor("wpack5", [128, 5, 2, D], bf, kind="ExternalInput")
    post_d = nc.dram_tensor("posT", [PD, nb, S], bf, kind="ExternalInput")
    pbf_d = nc.dram_tensor("packbf", [128, 536], bf, kind="ExternalInput")
    out_d = nc.dram_tensor("out", [nb, S, D], fp32, kind="ExternalOutput")

    # head h -> score-slot (spreads concurrent row-group MMs over banks)
    slot = [2 * (h % 4) + h // 4 for h in range(H)]

    with TileContext(nc) as tc:
        with (
            tc.tile_pool(name="wsb", bufs=1) as wsb,
            tc.tile_pool(name="xin", bufs=4) as xin,
            tc.tile_pool(name="qkv", bufs=2) as qkv,
            tc.tile_pool(name="esb", bufs=4) as esb,
            tc.tile_pool(name="bld", bufs=2) as bld,
            tc.tile_pool(name="small", bufs=2) as small,
            tc.tile_pool(name="ps", bufs=1, space="PSUM") as ps,
        ):
            scr_sb = wsb.tile([1, 1], fp32, tag="scr")
            scr2_sb = wsb.tile([1, 1], fp32, tag="scr2")
            nc.vector.memset(scr_sb, 0.0)
            nc.scalar.activation(scr2_sb, scr_sb, Exp)
            wu_sb = wsb.tile([128, 128], bf, tag="wu")
            nc.vector.memset(wu_sb, 0.0)
            wu_rhs = bass.AP(tensor=wu_sb.tensor, offset=wu_sb.offset,
                             ap=list(wu_sb.ap[:1]) + [[0, 4], [1, 128]])
            for i in range(5):
                wu_ps = ps.tile([128, 512], fp32, tag="scA")
                nc.tensor.matmul(wu_ps[0:128, 0:512], lhsT=wu_sb,
                                 rhs=wu_rhs, start=True, stop=True)
            # ---- packed resident weights (posT tiny, consts, then x/wp5) ----
            posT_sb = wsb.tile([PD, nb, S], bf, tag="posT")
            nc.sync.dma_start(out=posT_sb, in_=post_d[:, :, :])
            pbf_sb = wsb.tile([128, 536], bf, tag="pbf")
            nc.sync.dma_start(out=pbf_sb, in_=pbf_d[:, :])
            id_sb = pbf_sb[:, 0:128]
            ones_sb = pbf_sb[:, 128:256]
            outb_sb = pbf_sb[0:1, 256:512]
            w1_sb = pbf_sb[0:PD, 512:520]
            hw2_sb = pbf_sb[0:PD, 520:528]
            b1bf_sb = pbf_sb[0:PD, 528:529]

            # x prefetch ring (3 deep)
            xts = {}

            def issue_x(b):
                if b >= nb:
                    return
                t = xin.tile([128, 2, S], bf, tag="xt", name=f"xt{b}")
                nc.sync.dma_start(
                    out=t, in_=xt_d[b].rearrange("(c p) s -> p c s", p=128))
                xts[b] = t

            issue_x(0)

            wp5_sb = wsb.tile([128, 5, 2, D], bf, tag="wp5")
            nc.sync.dma_start(out=wp5_sb, in_=wp5_d[:, :, :, :])
            wq_sb = wp5_sb[:, 0]
            wk_sb = wp5_sb[:, 1]
            vt_sb = wp5_sb[:, 2]
            owt_sb = wp5_sb[:, 3]
            owtg_sb = wp5_sb[:, 4]

            issue_x(1)
            issue_x(2)

            # ---- pos branch part 1: h1 + transposed r + per-(b,sc) cols ----
            b1f_sb = wsb.tile([PD, 1], fp32, tag="b1f")
            nc.vector.tensor_copy(b1f_sb, b1bf_sb)
            h1_sb = wsb.tile([PD, nb, S], bf, tag="h1")
            rT_ps = ps.tile([128, nb, 2, H], fp32, tag="ctx")
            for b0 in range(0, nb, 2):
                w = min(2, nb - b0)
                h1_ps = ps.tile([PD, 512], fp32, tag="work", bufs=2)
                nc.tensor.matmul(
                    h1_ps[:, 0:256 * w], lhsT=w1_sb,
                    rhs=posT_sb[:, b0:b0 + w, :].rearrange("i b s -> i (b s)"),
                    start=True, stop=True)
                nc.vector.tensor_scalar(
                    out=h1_sb[:, b0:b0 + w, :].rearrange("i b s -> i (b s)"),
                    in0=h1_ps[:, 0:256 * w], scalar1=b1f_sb, scalar2=0.0,
                    op0=mybir.AluOpType.add, op1=mybir.AluOpType.max)
                for k in range(w):
                    for c in range(2):
                        nc.tensor.matmul(
                            rT_ps[:, b0 + k, c, :],
                            lhsT=h1_sb[:, b0 + k, 128 * c:128 * (c + 1)],
                            rhs=hw2_sb, start=True, stop=True)

            wcol_sb = wsb.tile([128, nb, 2, H], bf, tag="wcol")
            wcoln_sb = wsb.tile([128, nb, 2, H], bf, tag="wcoln")

            def pos_tail():
                # w-tilde = exp(-r) (softmax-invariant shifts dropped)
                nc.scalar.activation(
                    wcol_sb.rearrange("p b c h -> p (b c h)"),
                    rT_ps.rearrange("p b c h -> p (b c h)"), Exp)
                ws_ps = ps.tile([1, nb * H], fp32, tag="work", bufs=2)
                for c in range(2):
                    nc.tensor.matmul(
                        ws_ps, lhsT=ones_sb[:, 0:1],
                        rhs=wcol_sb[:, :, c, :],
                        start=(c == 0), stop=(c == 1))
                ws_sb = small.tile([1, nb * H], bf, tag="ws")
                nc.vector.tensor_copy(ws_sb, ws_ps)
                rb_ps = ps.tile([128, nb * H], fp32, tag="work", bufs=2)
                nc.tensor.matmul(rb_ps, lhsT=ones_sb[0:1, :], rhs=ws_sb,
                                 start=True, stop=True)
                rb_sb = wsb.tile([128, nb * H], fp32, tag="rb")
                nc.vector.reciprocal_approx_fast(rb_sb, rb_ps)
                rb_bc = bass.AP(
                    tensor=rb_sb.tensor, offset=rb_sb.offset,
                    ap=list(rb_sb.ap[:1]) + [[H, nb], [0, 2], [1, H]])
                nc.vector.tensor_mul(
                    wcoln_sb.rearrange("p b c h -> p (b c h)"),
                    wcol_sb.rearrange("p b c h -> p (b c h)"), rb_bc)

            # ---- main loop: skew-2 rounds ----
            # round b: [b1(b-2)] q(b) [b2] k(b) sc0(b) [b3] v(b) sc1(b) [b4]
            st = {}

            def a_q(b):
                xt_bf = xts[b]
                issue_x(b + 3)
                q_ps = ps.tile([128, 2, S], fp32, tag="work", bufs=2)
                for cm in range(2):
                    for ci in range(2):
                        nc.tensor.matmul(
                            q_ps[:, cm, :],
                            lhsT=wq_sb[:, ci, 128 * cm:128 * (cm + 1)],
                            rhs=xt_bf[:, ci, :],
                            start=(ci == 0), stop=(ci == 1))
                qT_sb = qkv.tile([128, 2, S], sdt, tag="q")
                nc.vector.tensor_copy(qT_sb, q_ps)
                st[("q", b)] = qT_sb

            def a_k(b):
                xt_bf = xts[b]
                k_ps = ps.tile([128, 2, S], fp32, tag="work", bufs=2)
                for cm in range(2):
                    for ci in range(2):
                        nc.tensor.matmul(
                            k_ps[:, cm, :],
                            lhsT=wk_sb[:, ci, 128 * cm:128 * (cm + 1)],
                            rhs=xt_bf[:, ci, :],
                            start=(ci == 0), stop=(ci == 1))
                kT_sb = qkv.tile([128, 2, S], sdt, tag="k")
                nc.vector.tensor_copy(kT_sb, k_ps)
                st[("k", b)] = kT_sb

            def a_sc(b, ct, half):
                # half 0: score-slots 0..3 -> tag scA; half 1: slots 4..7 -> scB
                qT_sb = st[("q", b)]
                kT_sb = st[("k", b)]
                if ct == 1 and half == 1:
                    st.pop(("q", b))
                    st.pop(("k", b))
                if half == 0:
                    e_sb = esb.tile([128, H, S], bf, tag="exp", bufs=6)
                    st.setdefault(("e", b), []).append(e_sb)
                else:
                    e_sb = st[("e", b)][ct]
                sc_ps = ps.tile([128, 4, S], fp32,
                                tag="scA" if half == 0 else "scB")
                hs = [h for h in range(H) if slot[h] // 4 == half]
                for h in hs:
                    rg = h % 4
                    so = slot[h] % 4
                    if fp8:
                        kap = kT_sb[32 * rg:32 * (rg + 1), h // 4,
                                    128 * ct:128 * (ct + 1)]
                        kap = bass.AP(
                            tensor=kap.tensor, offset=kap.offset,
                            ap=list(kap.ap[:1]) + [[0, 2]] + list(kap.ap[1:]))
                        qap = qT_sb[32 * rg:32 * (rg + 1), h // 4, :]
                        qap = bass.AP(
                            tensor=qap.tensor, offset=qap.offset,
                            ap=list(qap.ap[:1]) + [[0, 2]] + list(qap.ap[1:]))
                        nc.tensor.matmul(
                            sc_ps[:, so, :], lhsT=kap, rhs=qap,
                            start=True, stop=True, perf_mode=DR,
                            tile_position=(32 * rg, 0))
                    else:
                        nc.tensor.matmul(
                            sc_ps[:, so, :],
                            lhsT=kT_sb[32 * rg:32 * (rg + 1), h // 4,
                                       128 * ct:128 * (ct + 1)],
                            rhs=qT_sb[32 * rg:32 * (rg + 1), h // 4, :],
                            start=True, stop=True,
                            tile_position=(32 * rg, 0))
                escale = float(SCALE) * (0.5 if fp8 else 1.0)
                nc.scalar.activation(
                    e_sb[:, 4 * half:4 * (half + 1), :], sc_ps, Exp,
                    scale=escale)

            def a_v(b):
                xt_bf = xts.pop(b)
                v_ps = ps.tile([128, 2, D], fp32, tag="work", bufs=2)
                for ct in range(2):
                    for ci in range(2):
                        nc.tensor.matmul(
                            v_ps[:, ct, :],
                            lhsT=xt_bf[:, ci, 128 * ct:128 * (ct + 1)],
                            rhs=vt_sb[:, ci, :],
                            start=(ci == 0), stop=(ci == 1))
                v_sb = qkv.tile([128, 2, D], bf, tag="v", bufs=3)
                nc.vector.tensor_copy(v_sb, v_ps)
                st[("v", b)] = v_sb

            def b1(b):
                v_sb = st[("v", b)]
                exp_c = st[("e", b)]
                ctx_ps = ps.tile([128, 2, H, HD], fp32, tag="ctx")
                for sc in range(2):
                    for h in range(H):
                        for ct in range(2):
                            nc.tensor.matmul(
                                ctx_ps[:, sc, h, :],
                                lhsT=exp_c[ct][:, slot[h],
                                               128 * sc:128 * (sc + 1)],
                                rhs=v_sb[:, ct, 32 * h:32 * (h + 1)],
                                start=(ct == 0), stop=(ct == 1))
                den_ps = ps.tile([128, 2, H], fp32, tag="work", bufs=2)
                for sc in range(2):
                    for h in range(H):
                        for ct in range(2):
                            nc.tensor.matmul(
                                den_ps[:, sc, h:h + 1],
                                lhsT=exp_c[ct][:, slot[h],
                                               128 * sc:128 * (sc + 1)],
                                rhs=ones_sb[:, 0:1],
                                start=(ct == 0), stop=(ct == 1))
                recip_sb = bld.tile([128, 2, H], fp32, tag="recip")
                nc.vector.reciprocal_approx_fast(recip_sb, den_ps)

                blendT_sb = bld.tile([128, 2, H, HD], bf, tag="blendT")
                r_bc = bass.AP(
                    tensor=recip_sb.tensor, offset=recip_sb.offset,
                    ap=list(recip_sb.ap[:1]) + [[H, 2], [1, H], [0, HD]])
                nc.vector.tensor_mul(
                    blendT_sb.rearrange("p c h j -> p (c h j)"),
                    ctx_ps.rearrange("p c h j -> p (c h j)"), r_bc)
                st[("bT", b)] = blendT_sb

                vb_ps = ps.tile([128, 2], fp32, tag="work", bufs=2)
                for h in range(H):
                    for ct in range(2):
                        nc.tensor.matmul(
                            vb_ps[32 * (h % 4):32 * (h % 4) + 32,
                                  h // 4:h // 4 + 1],
                            lhsT=v_sb[:, ct, 32 * h:32 * (h + 1)],
                            rhs=wcoln_sb[:, b, ct, h:h + 1],
                            start=(ct == 0), stop=(ct == 1),
                            tile_position=(0, 32 * (h % 4)))
                vb_bf = small.tile([128, 2], bf, tag="vbf")
                nc.vector.tensor_copy(vb_bf, vb_ps)
                st[("vb", b)] = vb_bf

            def b2(b):
                blendT_sb = st.pop(("bT", b))
                bt_ps = ps.tile([128, 2, 2, 128], bf, tag="ctx")
                for sc in range(2):
                    for jq in range(2):
                        nc.tensor.transpose(
                            bt_ps[:, jq, sc, :],
                            blendT_sb[:, sc, 4 * jq:4 * (jq + 1), :].rearrange(
                                "p h j -> p (h j)"),
                            id_sb)
                blend_sb = bld.tile([128, 2, S], bf, tag="blend")
                nc.vector.tensor_copy(
                    blend_sb.rearrange("p c s -> p (c s)"),
                    bt_ps.rearrange("p a b s -> p (a b s)"))
                st[("bl", b)] = blend_sb

            def b3(b):
                blend_sb = st.pop(("bl", b))
                vb_bf = st.pop(("vb", b))
                st.pop(("v", b))
                st.pop(("e", b))
                f_ps = ps.tile([128, 2, D], fp32, tag="final")
                for sc in range(2):
                    nc.tensor.matmul(f_ps[:, sc, :], lhsT=ones_sb[0:1, :],
                                     rhs=outb_sb, start=True, stop=False)
                    for jq in range(2):
                        vb = vb_bf[:, jq:jq + 1]
                        vb_bcast = bass.AP(
                            tensor=vb.tensor, offset=vb.offset,
                            ap=list(vb.ap[:1]) + [[0, 128]])
                        nc.tensor.matmul(
                            f_ps[:, sc, :], lhsT=vb_bcast,
                            rhs=owtg_sb[:, jq, :], start=False, stop=False)
                        nc.tensor.matmul(
                            f_ps[:, sc, :],
                            lhsT=blend_sb[:, jq, 128 * sc:128 * (sc + 1)],
                            rhs=owt_sb[:, jq, :], start=False,
                            stop=(jq == 1))
                st[("f", b)] = f_ps

            def b4(b):
                f_ps = st.pop(("f", b))
                o_sb = bld.tile([128, 2, D], fp32, tag="o")
                if b == nb - 1:
                    for sc in range(2):
                        nc.vector.tensor_copy(o_sb[:, sc], f_ps[:, sc])
                        nc.sync.dma_start(
                            out=out_d[b, 128 * sc:128 * (sc + 1)].rearrange(
                                "(c p) d -> p c d", p=128),
                            in_=o_sb[:, sc])
                elif b == nb - 2:
                    nc.vector.tensor_copy(o_sb, f_ps)
                    nc.sync.dma_start(
                        out=out_d[b].rearrange("(c p) d -> p c d", p=128),
                        in_=o_sb)
                else:
                    nc.vector.tensor_copy(o_sb, f_ps)
                    nc.gpsimd.dma_start(
                        out=out_d[b].rearrange("(c p) d -> p c d", p=128),
                        in_=o_sb)

            def round_(b):
                bb = b - 2
                if bb >= 0:
                    b1(bb)
                if b < nb:
                    a_q(b)
                if bb >= 0:
                    b2(bb)
                if b < nb:
                    a_k(b)
                    a_sc(b, 0, 0)
                    a_sc(b, 0, 1)
                if bb >= 0:
                    b3(bb)
                if b < nb:
                    a_v(b)
                    a_sc(b, 1, 0)
                    a_sc(b, 1, 1)
                if bb >= 0:
                    b4(bb)

            round_(0)
            pos_tail()
            for b in range(1, nb):
                round_(b)
            # interleaved 2-batch tail drain
            if nb >= 2:
                b1(nb - 2)
                b2(nb - 2)
                b1(nb - 1)
                b3(nb - 2)
                b2(nb - 1)
                b4(nb - 2)
                b3(nb - 1)
                b4(nb - 1)
            else:
                b1(0)
                b2(0)
                b3(0)
                b4(0)

    nc.finalize()
    return nc


def _prep_inputs(inputs, nb=NB):
    g = 1.0 / (1.0 + np.exp(-inputs["gate"].astype(np.float64)))
    g = g.astype(np.float32)  # [H]
    omg_j = np.repeat(1.0 - g, HD)  # per j = 32h+d'
    gr_j = np.repeat(g / (1.0 - g), HD)

    wqT = inputs["Wq"].T.astype(np.float32)
    wkT = inputs["Wk"].T.astype(np.float32)
    vT = inputs["v_embed"].reshape(D, D).T * omg_j[None, :]
    owT = inputs["out_w"].T.astype(np.float32)
    owTg = inputs["out_w"].T * gr_j[:, None]

    wp5 = np.zeros((128, 5, 2, D), dtype=np.float32)
    for i, w in enumerate((wqT, wkT, vT, owT, owTg)):
        wp5[:, i] = w.reshape(2, 128, D).transpose(1, 0, 2)
    wp5 = wp5.astype(bf16)

    pbf = np.zeros((128, 536), dtype=np.float32)
    pbf[:, 0:128] = np.eye(128, dtype=np.float32)
    pbf[:, 128:256] = 1.0
    pbf[0, 256:512] = inputs["out_b"].astype(np.float32)
    pbf[0:PD, 512:520] = inputs["pos_w1"].T.astype(np.float32)
    # hw2[i, h] = -(head_w @ pos_w2)[h, i]
    hw2 = -(inputs["head_w"].astype(np.float64)
            @ inputs["pos_w2"].astype(np.float64)).T
    pbf[0:PD, 520:528] = hw2.astype(np.float32)
    pbf[0:PD, 528] = inputs["pos_b1"].astype(np.float32)
    pbf = pbf.astype(bf16)

    x = np.asarray(inputs["x"], dtype=np.float32)
    pos = np.asarray(inputs["pos"], dtype=np.float32)
    xT = np.ascontiguousarray(x.transpose(0, 2, 1)).astype(bf16)  # [B, D, S]
    posT = np.ascontiguousarray(pos.transpose(2, 0, 1)).astype(bf16)  # [PD,B,S]
    ncores = B // nb
    in_maps = []
    for c in range(ncores):
        m = dict(wpack5=wp5, packbf=pbf)
        m["posT"] = np.ascontiguousarray(posT[:, c * nb:(c + 1) * nb])
        m["xT"] = np.ascontiguousarray(xT[c * nb:(c + 1) * nb])
        in_maps.append(m)
    return in_maps


def kernel(**inputs):
    from concourse.bass_utils import run_bass_kernel_spmd

    inputs = {k: np.asarray(v) for k, v in inputs.items()}
    if "nc" not in _CACHE:
        _CACHE["nc"] = _build(NB)
    in_maps = _prep_inputs(inputs)
    res = run_bass_kernel_spmd(_CACHE["nc"], in_maps, core_ids=list(range(NCORES)))
    out = np.concatenate([r["out"] for r in res.results], axis=0)
    return out.astype(np.float32)
